# revision 1
# baseline (speedup 1.0000x reference)
"""GAT layer kernel for Trainium2, 8-core data-parallel over batch.

Math (per batch b, head h):
    h = x @ W                              [N, H*HD]
    s_n = <h[n, h*HD:(h+1)*HD], a_src[h]>  t_n likewise with a_dst
    A[j, i] = exp(leakyrelu(s_i + t_j, 0.2))
            = max(e^{t_j} * e^{s_i}, e^{0.2 t_j} * e^{0.2 s_i})   (exact identity)
    out[i]  = (sum_j A[j, i] * h_j) / (sum_j A[j, i])
No row-max subtraction is needed: max(s_i + t_j) ~ 51 for these inputs, and
exp(51) ~ 1.4e22 is far below the fp32/bf16 overflow threshold; softmax ratios
are scale-invariant so unnormalized exponentials are numerically fine.

Per core (= one batch element):
  - hT/h_node/s/t via small float32r matmuls (fp32 is 1/4 rate on PE)
  - A tiles [j, i] (j on partitions) built per (head, j-tile) by one of:
      'act': E = Prelu(S_bcast + t_col, alpha=0.2) ; A = Exp(E)      (2 ACT ops)
      'dve': R2 = Es02_bcast * Et02_col ; A = max(Es_bcast * Et_col, R2)
             (tensor_scalar + scalar_tensor_tensor, bf16, 2 DVE ops)
    The row-broadcast tensors come from DMA partition-broadcasts split over
    three DMA lanes (sync / gpsimd / tensor queues).
  - out^T[(h,d)+Z, i] accumulated in PSUM: lhsT = [h_node | ones] block, rhs = A
  - normalize by the Z row: fold Z into [128, NT] columns via DRAM (the DVE
    reciprocal is ~8 cyc/elem serial along the free dim, so a [1, N] row costs
    8.5us but [128, NT] is ~0.1us), unfold, K=1-matmul-broadcast, multiply.
  - engines have strict-FIFO instruction streams, so every op that waits on a
    whole head's matmul output is emitted with a one-head lag (or after the
    bulk loop) to avoid stalling the producers.
  - host transposes out^T back to node-major when unsharding.
"""

import numpy as np

B, N, IN_F, OUT_F, H = 8, 1024, 128, 128, 4
HD = OUT_F // H  # 32
NEG = 0.2
N_CORES = 8
NT = N // 128  # 8 node tiles


def _default_schemes():
    # Interleave within each head so ScalarE and VectorE stay busy together.
    # interleave within each head so ScalarE and VectorE stay busy together
    # AND the PE consumes tiles at a steady mixed pace; 14 act / 18 dve.
    sch = {(h, jt): ("act" if jt in (0, 2, 4) else "dve")
           for h in range(H) for jt in range(NT)}
    sch[(0, 6)] = "act"
    sch[(2, 6)] = "act"
    return sch


SCHEMES = _default_schemes()

A_DT = "bfloat16"  # dtype of the attention tiles + matmul weights

_CACHE = {}


def _build_nc():
    import concourse.bacc as bacc
    import concourse.tile as tile
    from concourse import mybir

    f32 = mybir.dt.float32
    f16 = mybir.dt.float16
    f32r = mybir.dt.float32r
    adt = getattr(mybir.dt, A_DT)
    AF = mybir.ActivationFunctionType
    ALU = mybir.AluOpType

    nc = bacc.Bacc("TRN2", target_bir_lowering=False, debug=False,
                   num_devices=N_CORES)

    xT = nc.declare_dram_parameter("xT", [IN_F, N], f32, isOutput=False)
    Wd = nc.declare_dram_parameter("W", [IN_F, OUT_F], f32, isOutput=False)
    Wa = nc.declare_dram_parameter("Wa", [IN_F, 2 * H], f32, isOutput=False)
    ind4_d = nc.declare_dram_parameter("ind4", [H, OUT_F], f32, isOutput=False)
    outT = nc.declare_dram_parameter("outT", [OUT_F, N], f32, isOutput=True)

    st_dram = nc.dram_tensor("st_scratch", [2 * H, N], f32)
    s16_dram = nc.dram_tensor("s16_scratch", [H, N], f16)
    es_dram = nc.dram_tensor("es_scratch", [H, N], adt)
    es02_dram = nc.dram_tensor("es02_scratch", [H, N], adt)
    z_dram = nc.dram_tensor("z_scratch", [H, N], f32)
    rz_dram = nc.dram_tensor("rz_scratch", [H, N], f32)

    with tile.TileContext(nc) as tc:
      with (
        tc.tile_pool(name="const", bufs=1) as cpool,
        tc.tile_pool(name="etile", bufs=4) as epool,
        tc.tile_pool(name="atile", bufs=16) as apool,
        tc.tile_pool(name="r2tile", bufs=4) as r2pool,
        tc.tile_pool(name="otile", bufs=1) as opool,
        tc.tile_pool(name="rztile", bufs=1) as rzpool,
      ):
        with tc.tile_pool(name="ps_pre", bufs=1, space="PSUM") as pspre:
            # ---- load inputs ----
            xT_sb = cpool.tile([IN_F, N], f32, tag="xT")
            nc.sync.dma_start(out=xT_sb[:, 0:512], in_=xT[:, 0:512])
            nc.gpsimd.dma_start(out=xT_sb[:, 512:N], in_=xT[:, 512:N])
            W_sb = cpool.tile([IN_F, OUT_F], f32, tag="W")
            nc.sync.dma_start(out=W_sb, in_=Wd[:])
            Wa_sb = cpool.tile([IN_F, 2 * H], f32, tag="Wa")
            nc.sync.dma_start(out=Wa_sb, in_=Wa[:])

            # fp32 matmul is 1/4 rate; float32r streams 1 col/cycle at N>=256
            # but needs explicitly rounded inputs (a convert copy).
            xTr = cpool.tile([IN_F, N], f32r, tag="xTr")
            nc.vector.tensor_copy(out=xTr, in_=xT_sb)
            Wr = cpool.tile([IN_F, OUT_F], f32r, tag="Wr")
            nc.vector.tensor_copy(out=Wr, in_=W_sb)
            War = cpool.tile([IN_F, 2 * H], f32r, tag="War")
            nc.vector.tensor_copy(out=War, in_=Wa_sb)

            # ---- st rows = (W @ a_ext)^T @ xT -> [2H, N]; the host
            # precomputes Wa = W @ a_ext so s/t skip the hT dependency ----
            st_ps = pspre.tile([2 * H, N], f32, tag="st")
            for c in range(2):
                nc.tensor.matmul(st_ps[:, 512 * c:512 * (c + 1)], War,
                                 xTr[:, 512 * c:512 * (c + 1)],
                                 start=True, stop=True)
            st_sb = cpool.tile([2 * H, N], f32, tag="st_sb")
            nc.vector.tensor_copy(out=st_sb, in_=st_ps)

            # ---- t columns via DRAM round trip ----
            nc.sync.dma_start(out=st_dram[:], in_=st_sb)
            # tc_all[p, h*NT+jt] = t_col for head h, j-tile jt
            tc_all = cpool.tile([128, H * NT], f32, tag="tc")
            nc.gpsimd.dma_start(
                out=tc_all,
                in_=st_dram[:].rearrange("h (jt p) -> p (h jt)", p=128)[
                    :, H * NT:],
            )

            # fp16 s rows for the ACT-path broadcasts (half the DMA bytes;
            # |s| < 40 so fp16 is safe, and its tiny rounding is a per-column
            # scale that cancels in the softmax). These and the exp'd s rows
            # are emitted before anything that needs the tc_all DRAM hop so
            # the broadcast chain starts as early as possible.
            s16_rows = cpool.tile([H, N], f16, tag="s16")
            nc.vector.tensor_copy(out=s16_rows, in_=st_sb[0:H, :])
            nc.sync.dma_start(out=s16_dram[:], in_=s16_rows)

            # exp'd s rows (bf16) for the DVE path broadcasts
            es_rows = cpool.tile([H, N], adt, tag="es_rows")
            nc.scalar.activation(out=es_rows, in_=st_sb[0:H, :], func=AF.Exp)
            nc.gpsimd.dma_start(out=es_dram[:], in_=es_rows)
            es02_rows = cpool.tile([H, N], adt, tag="es02_rows")
            nc.scalar.activation(out=es02_rows, in_=st_sb[0:H, :], func=AF.Exp,
                                 scale=NEG)
            nc.sync.dma_start(out=es02_dram[:], in_=es02_rows)

            # exp'd per-partition scalars for the DVE path
            etc = cpool.tile([128, H * NT], f32, tag="etc")
            nc.scalar.activation(out=etc, in_=tc_all, func=AF.Exp)
            etc02 = cpool.tile([128, H * NT], f32, tag="etc02")
            nc.scalar.activation(out=etc02, in_=tc_all, func=AF.Exp, scale=NEG)

            # indicator weights: ind[k, 32h+d] = (k == h) — used to broadcast
            # row h of a [4, N] tile across 32 output partitions via K=4 matmul
            ind4_f = cpool.tile([H, OUT_F], f32, tag="ind4f")
            nc.sync.dma_start(out=ind4_f, in_=ind4_d[:])
            ind4 = cpool.tile([H, OUT_F], f32r, tag="ind4")
            nc.vector.tensor_copy(out=ind4, in_=ind4_f)

            # ---- weight tiles: wt[:, 132jt+33h : +32] = h_node + a 1s col ----
            hn_ps = pspre.tile([128, N], f32, tag="hn")
            for jt in range(NT):
                nc.tensor.matmul(hn_ps[:, 128 * jt:128 * (jt + 1)],
                                 xTr[:, 128 * jt:128 * (jt + 1)], Wr,
                                 start=True, stop=True)
            wt_all = cpool.tile([128, NT * 33 * H], adt, tag="wt")
            wt_v = wt_all[:].rearrange("p (jt h c) -> p jt h c", h=H, c=33)
            nc.vector.tensor_copy(
                out=wt_v[:, :, :, 0:32],
                in_=hn_ps[:].rearrange("p (jt h c) -> p jt h c", h=H, c=32))
            nc.vector.memset(wt_v[:, :, :, 32:33], 1.0)
            wts = [wt_all[:, 132 * jt:132 * (jt + 1)] for jt in range(NT)]

        # ---- broadcast tiles per head, spread over three DMA lanes ----
        s_bcast, es_b, es02_b = {}, {}, {}
        for h in range(H):
            sb = cpool.tile([128, N], f16, tag=f"sb{h}")
            nc.sync.dma_start(
                out=sb, in_=s16_dram[h:h + 1, :].to_broadcast([128, N]))
            s_bcast[h] = sb
            eb2 = cpool.tile([128, N], adt, tag=f"es02b{h}")
            nc.sync.dma_start(
                out=eb2, in_=es02_dram[h:h + 1, :].to_broadcast([128, N]))
            es02_b[h] = eb2
            eb = cpool.tile([128, N], adt, tag=f"esb{h}")
            nc.gpsimd.dma_start(
                out=eb, in_=es_dram[h:h + 1, :].to_broadcast([128, N]))
            es_b[h] = eb

        # ---- main loop: oh bufs=4 keeps all four heads' accumulators
        # resident (8 PSUM banks) so no matmul ever waits on a slot release
        with tc.tile_pool(name="ps_main", bufs=4, space="PSUM") as psmain:
            ohs = [None] * H
            for h in range(H):
                oh = psmain.tile([33, N], f32, tag="oh")
                ohs[h] = oh
                for jt in range(NT):
                    idx = h * NT + jt
                    if SCHEMES[(h, jt)] == "act":
                        e_t = epool.tile([128, N], f32, tag="et")
                        nc.scalar.activation(out=e_t, in_=s_bcast[h],
                                             func=AF.Prelu,
                                             bias=tc_all[:, idx:idx + 1],
                                             scale=1.0, alpha=NEG)
                        a_t = apool.tile([128, N], adt, tag="at")
                        nc.scalar.activation(out=a_t, in_=e_t, func=AF.Exp)
                    else:
                        r2 = r2pool.tile([128, N], adt, tag="r2")
                        nc.vector.tensor_scalar_mul(
                            out=r2, in0=es02_b[h],
                            scalar1=etc02[:, idx:idx + 1])
                        a_t = apool.tile([128, N], adt, tag="at")
                        nc.vector.scalar_tensor_tensor(
                            out=a_t, in0=es_b[h], scalar=etc[:, idx:idx + 1],
                            in1=r2, op0=ALU.mult, op1=ALU.max)
                    for c in range(2):
                        nc.tensor.matmul(
                            oh[:, 512 * c:512 * (c + 1)],
                            wts[jt][:, 33 * h:33 * (h + 1)],
                            a_t[:, 512 * c:512 * (c + 1)],
                            start=(jt == 0), stop=(jt == NT - 1))
            # Scheduler-time floor: without it the Tile scheduler's cost
            # model (which underestimates DMA latency) interleaves these
            # tail ops ahead of bulk ops in the strict-FIFO engine streams,
            # stalling the producers for tens of us.
            tail_ctx = tc.tile_wait_until(0.2)
            tail_ctx.__enter__()
            # ---- deferred normalize tail. 1/Z via integer-magic seed +
            # 2 Newton-Raphson iterations on DVE, batched [H, N] for all
            # heads: ~7us of engine ops with a single small-DMA stage (the
            # Z-row assembly) instead of 4 DMA round-trips per head whose
            # ~5us/hop latency dominated earlier revisions. Z in [6e-6, 1e25]
            # is comfortably inside the magic-constant seed's valid range.
            ocps = []
            for h in range(H):
                ocp = opool.tile([33, N], f32, tag=f"ocp{h}")
                if h < 2:
                    nc.scalar.copy(out=ocp, in_=ohs[h])
                else:
                    nc.vector.tensor_copy(out=ocp, in_=ohs[h])
                ocps.append(ocp)
            # assemble Z rows in one [H, N] tile (4 small on-chip DMAs),
            # then 1/Z = int-magic seed + one Newton iteration (~0.3% seed^2
            # error, far inside the tolerance), f32r-rounded on the last op
            z4 = rzpool.tile([H, N], f32, tag="z4")
            for h in range(H):
                nc.sync.dma_start(out=z4[h:h + 1, :], in_=ocps[h][32:33, :])
            i32 = mybir.dt.int32
            ynot = rzpool.tile([H, N], f32, tag="ynot")
            nc.vector.tensor_scalar(
                out=ynot[:].bitcast(i32), in0=z4[:].bitcast(i32),
                scalar1=0xFFFFFFFF - (1 << 32), scalar2=None,
                op0=ALU.bitwise_xor)
            y = rzpool.tile([H, N], f32, tag="y")
            nc.vector.tensor_scalar(
                out=y[:].bitcast(i32), in0=ynot[:].bitcast(i32),
                scalar1=0x7EF311C4, scalar2=None, op0=ALU.add)
            m = rzpool.tile([H, N], f32, tag="nr_m")
            nc.vector.tensor_tensor(out=m, in0=z4, in1=y, op=ALU.mult)
            s2 = rzpool.tile([H, N], f32, tag="nr_s")
            nc.vector.tensor_scalar(out=s2, in0=m, scalar1=2.0,
                                    scalar2=-1.0, op0=ALU.subtract,
                                    op1=ALU.mult)
            rz4 = rzpool.tile([H, N], f32r, tag="rz4")
            nc.vector.tensor_tensor(out=rz4, in0=y, in1=s2, op=ALU.mult)
        with tc.tile_pool(name="ps_norm", bufs=2, space="PSUM") as psnorm:
            rzbs = []
            for h in range(H):
                rzb = psnorm.tile([HD, N], f32, tag="rzb")
                for c in range(2):
                    nc.tensor.matmul(rzb[:, 512 * c:512 * (c + 1)],
                                     ind4[:, HD * h:HD * (h + 1)],
                                     rz4[:, 512 * c:512 * (c + 1)],
                                     start=True, stop=True)
                rzbs.append(rzb)
                o_sb = opool.tile([HD, N], f32, tag=f"osb{h}")
                nc.vector.tensor_tensor(out=o_sb, in0=ocps[h][0:HD, :],
                                        in1=rzb, op=ALU.mult)
                nc.sync.dma_start(out=outT[HD * h:HD * (h + 1), :],
                                  in_=o_sb)
            tail_ctx.__exit__(None, None, None)

    nc.compile()
    return nc


def _get_nc():
    if "nc" not in _CACHE:
        _CACHE["nc"] = _build_nc()
    return _CACHE["nc"]


def kernel(x, W, a_src, a_dst):
    from concourse.bass_utils import run_bass_kernel_spmd

    x = np.asarray(x, dtype=np.float32)
    W = np.asarray(W, dtype=np.float32)
    a_src = np.asarray(a_src, dtype=np.float32)
    a_dst = np.asarray(a_dst, dtype=np.float32)

    a_ext = np.zeros((OUT_F, 2 * H), np.float32)
    ind4 = np.zeros((H, OUT_F), np.float32)
    for h in range(H):
        a_ext[h * HD:(h + 1) * HD, h] = a_src[h]
        a_ext[h * HD:(h + 1) * HD, H + h] = a_dst[h]
        ind4[h, h * HD:(h + 1) * HD] = 1.0
    Wa = W @ a_ext

    nc = _get_nc()
    in_maps = [
        {"xT": np.ascontiguousarray(x[c].T), "W": W, "Wa": Wa,
         "ind4": ind4}
        for c in range(N_CORES)
    ]
    res = run_bass_kernel_spmd(nc, in_maps, core_ids=list(range(N_CORES)))
    out = np.stack([res.results[c]["outT"].T for c in range(N_CORES)], axis=0)
    return np.ascontiguousarray(out, dtype=np.float32)



# revision 8
# speedup vs baseline: 1.2048x; 1.2048x over previous
"""GAT layer kernel for Trainium2, 8-core data-parallel over batch.

Math (per batch b, head h):
    h = x @ W                              [N, H*HD]
    s_i = <h_i, a_src[h]>,  t_j = <h_j, a_dst[h]>
    A[j, i] = exp(leakyrelu(s_i + t_j, 0.2))
    out[i]  = (sum_j A[j, i] * h_j) / (sum_j A[j, i])

Softmax over j is invariant to any per-column (per-i) scaling, so scale
column i by e^{-s_i}:
    Ā[j, i] = e^{-s_i} A[j, i] = max(e^{t_j}, e^{0.2 t_j} * e^{-0.8 s_i})
(exact: for s+t>=0 the left branch wins and equals e^{s+t-s}; below, the
right branch.) The left branch has no i-dependence, so each [128, N]
attention tile is ONE DVE tensor_scalar op with two per-partition scalars:
    a = (es08_bcast * etc02_col) max etc_col
in bf16 (2x DVE rate). The only broadcast tensor is es08[h] = e^{-0.8 s},
one per head, built by a K=1 PE matmul (ones ⊗ s row) into PSUM and an
ACT exp into SBUF — no DRAM broadcast round-trips.

t is produced directly in column form (t[j] on partitions) by
matmul(xT_tile, W @ a_dst) per node tile, skipping any transpose hop.

Aggregation: out^T[(h,d)|Z, i] accumulated in PSUM with a [h_node | ones]
weight block (Z row = column sums of Ā). Per-head normalize tail is
pipelined one head behind the accumulation: Z row -> DRAM fold to
[128, 8] -> DVE reciprocal -> unfold -> DMA partition-broadcast to
[32, N] -> multiply -> f16 DMA out. Host transposes/casts on unshard.
"""

import numpy as np

B, N, IN_F, OUT_F, H = 8, 1024, 128, 128, 4
HD = OUT_F // H  # 32
NEG = 0.2
N_CORES = 8
NT = N // 128  # 8 node tiles

A_DT = "bfloat16"

_CACHE = {}


def _build_nc():
    import concourse.bacc as bacc
    import concourse.tile as tile
    from concourse import mybir

    f32 = mybir.dt.float32
    f16 = mybir.dt.float16
    f32r = mybir.dt.float32r
    adt = getattr(mybir.dt, A_DT)
    AF = mybir.ActivationFunctionType
    ALU = mybir.AluOpType

    nc = bacc.Bacc("TRN2", target_bir_lowering=False, debug=False,
                   num_devices=N_CORES)

    xT = nc.declare_dram_parameter("xT", [IN_F, N], f32, isOutput=False)
    Wd = nc.declare_dram_parameter("W", [IN_F, OUT_F], f32, isOutput=False)
    Wa = nc.declare_dram_parameter("Wa", [IN_F, 2 * H], f32, isOutput=False)
    # Wsb[:, 128h:128(h+1)] = (W @ a_src[h]) ⊗ ones(128): rank-1 weights so
    # matmul(Wsb_h, xTr) emits s_h[i] broadcast across all 128 partitions.
    Wsb = nc.declare_dram_parameter("Wsb", [IN_F, H * 128], f32,
                                    isOutput=False)
    outT = nc.declare_dram_parameter("outT", [OUT_F, N], f16, isOutput=True)

    z_dram = nc.dram_tensor("z_scratch", [H, N], f32)
    rz_dram = nc.dram_tensor("rz_scratch", [H, N], f32)

    with tile.TileContext(nc) as tc:
      with (
        tc.tile_pool(name="const", bufs=1) as cpool,
        tc.tile_pool(name="atile", bufs=8) as apool,
        tc.tile_pool(name="otile", bufs=2) as opool,
        tc.tile_pool(name="ztile", bufs=2) as zpool,
      ):
        # ---- load inputs ----
        xT_sb = cpool.tile([IN_F, N], f32, tag="xT")
        nc.sync.dma_start(out=xT_sb[:, 0:512], in_=xT[:, 0:512])
        nc.gpsimd.dma_start(out=xT_sb[:, 512:N], in_=xT[:, 512:N])
        W_sb = cpool.tile([IN_F, OUT_F], f32, tag="W")
        nc.gpsimd.dma_start(out=W_sb, in_=Wd[:])
        Wa_sb = cpool.tile([IN_F, 2 * H], f32, tag="Wa")
        nc.sync.dma_start(out=Wa_sb, in_=Wa[:])
        Wsb_sb = cpool.tile([IN_F, H * 128], f32, tag="Wsb")
        nc.sync.dma_start(out=Wsb_sb, in_=Wsb[:])

        # fp32 matmul is 1/4 rate; float32r streams 1 col/cycle but needs
        # explicitly rounded inputs (a convert copy).
        xTr = cpool.tile([IN_F, N], f32r, tag="xTr")
        nc.vector.tensor_copy(out=xTr, in_=xT_sb)
        Wr = cpool.tile([IN_F, OUT_F], f32r, tag="Wr")
        nc.vector.tensor_copy(out=Wr, in_=W_sb)
        War = cpool.tile([IN_F, 2 * H], f32r, tag="War")
        nc.vector.tensor_copy(out=War, in_=Wa_sb)
        Wsbr = cpool.tile([IN_F, H * 128], f32r, tag="Wsbr")
        nc.vector.tensor_copy(out=Wsbr, in_=Wsb_sb)

        # prime the ACT exp table before it's on the critical path
        warm = cpool.tile([1, 8], f32, tag="warm")
        nc.scalar.activation(out=warm, in_=Wa_sb[0:1, 0:8], func=AF.Exp)

        # ---- t columns: tc_ps[j, 4*jt + h] = t_h[128*jt + j] ----
        with tc.tile_pool(name="ps_tc", bufs=1, space="PSUM") as pstc:
            tc_ps = pstc.tile([128, H * NT], f32, tag="tc")
            for jt in range(NT):
                nc.tensor.matmul(tc_ps[:, H * jt:H * (jt + 1)],
                                 xTr[:, 128 * jt:128 * (jt + 1)],
                                 War[:, H:2 * H], start=True, stop=True)
            etc = cpool.tile([128, H * NT], f32, tag="etc")
            nc.scalar.activation(out=etc, in_=tc_ps, func=AF.Exp)
            etc02 = cpool.tile([128, H * NT], f32, tag="etc02")
            nc.scalar.activation(out=etc02, in_=tc_ps, func=AF.Exp, scale=NEG)

        # ---- es08_b[h][p, i] = e^{-0.8 s_h[i]}: rank-1-weight PE broadcast
        # matmul + ACT exp. bufs=1 serializes heads through one 2-bank slot;
        # later heads are emitted interleaved behind the main loop's matmuls.
        pssb = tc.tile_pool(name="ps_sb", bufs=1, space="PSUM")
        sbpool = pssb.__enter__()
        es08_b = {}

        def emit_sbcast(h):
            sb_ps = sbpool.tile([128, N], f32, tag="sb")
            for c in range(2):
                nc.tensor.matmul(sb_ps[:, 512 * c:512 * (c + 1)],
                                 Wsbr[:, 128 * h:128 * (h + 1)],
                                 xTr[:, 512 * c:512 * (c + 1)],
                                 start=True, stop=True)
            eb = cpool.tile([128, N], adt, tag=f"es08b{h}")
            nc.scalar.activation(out=eb, in_=sb_ps, func=AF.Exp, scale=-0.8)
            es08_b[h] = eb

        emit_sbcast(0)

        # ---- weight tiles: wt[:, 132jt+33h : +32] = h_node, col 32 = 1s ----
        wt_all = cpool.tile([128, NT * 33 * H], adt, tag="wt")
        wt_v = wt_all[:].rearrange("p (jt h c) -> p jt h c", h=H, c=33)
        nc.vector.memset(wt_v[:, :, :, 32:33], 1.0)
        with tc.tile_pool(name="ps_hn", bufs=1, space="PSUM") as pshn:
            hn_ps = pshn.tile([128, N], f32, tag="hn")
            for jt in range(NT):
                nc.tensor.matmul(hn_ps[:, 128 * jt:128 * (jt + 1)],
                                 xTr[:, 128 * jt:128 * (jt + 1)], Wr,
                                 start=True, stop=True)
                nc.vector.tensor_copy(
                    out=wt_v[:, jt, :, 0:32],
                    in_=hn_ps[:, 128 * jt:128 * (jt + 1)].rearrange(
                        "p (h c) -> p h c", c=32))
        wts = [wt_all[:, 132 * jt:132 * (jt + 1)] for jt in range(NT)]

        emit_sbcast(1)

        # ---- main loop + pipelined per-head tail ----
        def emit_tail(h, oh):
            # Z row -> DRAM fold [128, 8] -> reciprocal -> unfold ->
            # partition-broadcast [32, N] -> multiply -> f16 out
            zrow = zpool.tile([1, N], f32, tag="zrow")
            nc.scalar.copy(out=zrow, in_=oh[32:33, :])
            q = nc.sync if h % 2 == 0 else nc.gpsimd
            q.dma_start(out=z_dram[h:h + 1, :], in_=zrow)
            zf = zpool.tile([128, NT], f32, tag="zf")
            q.dma_start(out=zf, in_=z_dram[h:h + 1, :].rearrange(
                "o (p c) -> (o p) c", p=128))
            rzf = zpool.tile([128, NT], f32, tag="rzf")
            nc.vector.reciprocal(out=rzf, in_=zf)
            q.dma_start(out=rz_dram[h:h + 1, :].rearrange(
                "o (p c) -> (o p) c", p=128), in_=rzf)
            rzb = zpool.tile([HD, N], f32, tag="rzb")
            q.dma_start(out=rzb, in_=rz_dram[h:h + 1, :].to_broadcast([HD, N]))
            o16 = opool.tile([HD, N], f16, tag="o16")
            nc.vector.tensor_tensor(out=o16, in0=oh[0:HD, :], in1=rzb,
                                    op=ALU.mult)
            q.dma_start(out=outT[HD * h:HD * (h + 1), :], in_=o16)

        with tc.tile_pool(name="ps_main", bufs=3, space="PSUM") as psmain:
            ohs = [None] * H
            for h in range(H):
                oh = psmain.tile([33, N], f32, tag="oh")
                ohs[h] = oh
                for jt in range(NT):
                    idx = H * jt + h
                    a_t = apool.tile([128, N], adt, tag="at")
                    nc.vector.tensor_scalar(
                        out=a_t, in0=es08_b[h],
                        scalar1=etc02[:, idx:idx + 1],
                        scalar2=etc[:, idx:idx + 1],
                        op0=ALU.mult, op1=ALU.max)
                    for c in range(2):
                        nc.tensor.matmul(
                            oh[:, 512 * c:512 * (c + 1)],
                            wts[jt][:, 33 * h:33 * (h + 1)],
                            a_t[:, 512 * c:512 * (c + 1)],
                            start=(jt == 0), stop=(jt == NT - 1))
                if h + 2 < H:
                    emit_sbcast(h + 2)
                if h >= 1:
                    emit_tail(h - 1, ohs[h - 1])
            emit_tail(H - 1, ohs[H - 1])
        pssb.__exit__(None, None, None)

    nc.compile()
    return nc


def _get_nc():
    if "nc" not in _CACHE:
        _CACHE["nc"] = _build_nc()
    return _CACHE["nc"]


def _prep_in_maps(x, W, a_src, a_dst):
    x = np.asarray(x, dtype=np.float32)
    W = np.asarray(W, dtype=np.float32)
    a_src = np.asarray(a_src, dtype=np.float32)
    a_dst = np.asarray(a_dst, dtype=np.float32)

    a_ext = np.zeros((OUT_F, 2 * H), np.float32)
    for h in range(H):
        a_ext[h * HD:(h + 1) * HD, h] = a_src[h]
        a_ext[h * HD:(h + 1) * HD, H + h] = a_dst[h]
    Wa = W @ a_ext
    Wsb = np.ascontiguousarray(np.repeat(Wa[:, 0:H], 128, axis=1))

    return [
        {"xT": np.ascontiguousarray(x[c].T), "W": W, "Wa": Wa, "Wsb": Wsb}
        for c in range(N_CORES)
    ]


def kernel(x, W, a_src, a_dst):
    from concourse.bass_utils import run_bass_kernel_spmd

    nc = _get_nc()
    in_maps = _prep_in_maps(x, W, a_src, a_dst)
    res = run_bass_kernel_spmd(nc, in_maps, core_ids=list(range(N_CORES)))
    out = np.stack([res.results[c]["outT"].T.astype(np.float32)
                    for c in range(N_CORES)], axis=0)
    return np.ascontiguousarray(out, dtype=np.float32)


# revision 9
# speedup vs baseline: 1.9046x; 1.5808x over previous
"""GAT layer kernel for Trainium2, 8-core data-parallel over batch.

Math (per batch b, head h):
    h = x @ W                              [N, H*HD]
    s_i = <h_i, a_src[h]>,  t_j = <h_j, a_dst[h]>
    A[j, i] = exp(leakyrelu(s_i + t_j, 0.2))
    out[i]  = (sum_j A[j, i] * h_j) / (sum_j A[j, i])

Softmax over j is invariant to any per-column (per-i) scaling, so scale
column i by e^{-s_i}:
    Ā[j, i] = e^{-s_i} A[j, i] = max(e^{t_j}, e^{0.2 t_j} * e^{-0.8 s_i})
(exact: for s+t>=0 the left branch wins and equals e^{s+t-s}; below, the
right branch.) The left branch has no i-dependence, so each [128, N]
attention tile is ONE DVE tensor_scalar op with two per-partition scalars:
    a = (es08_bcast * etc02_col) max etc_col
in bf16 (2x DVE rate). The only broadcast tensor is es08[h] = e^{-0.8 s},
one per head, built by a rank-1-weight PE matmul (each weight column =
W @ a_src[h], via a stride-0 AP) into PSUM and an ACT exp into SBUF — no
DRAM broadcast round-trips.

t is produced directly in column form (t[j] on partitions) by
matmul(xT_tile, W @ a_dst) per node tile, skipping any transpose hop.

All PE inputs are f16 (full 1 col/cycle rate, half the DMA bytes of
f32r, ~0.05% rounding — well under the bf16 noise of the A tiles).

Aggregation: out^T[(h,d)|Z, i] accumulated in PSUM with a [h_node | ones]
weight block (Z row = column sums of Ā). Per head, the raw [33, N]
accumulator (numerator rows + Z row) is copied to SBUF by the otherwise
idle ACT engine and DMA'd out; the division by Z, the transpose back to
node-major, and the f32 cast happen on the host during unsharding.
"""

import numpy as np

B, N, IN_F, OUT_F, H = 8, 1024, 128, 128, 4
HD = OUT_F // H  # 32
NEG = 0.2
N_CORES = 8
NT = N // 128  # 8 node tiles

_CACHE = {}


def _build_nc():
    import concourse.bacc as bacc
    import concourse.tile as tile
    from concourse import mybir

    f32 = mybir.dt.float32
    f16 = mybir.dt.float16
    bf16 = mybir.dt.bfloat16
    AF = mybir.ActivationFunctionType
    ALU = mybir.AluOpType

    nc = bacc.Bacc("TRN2", target_bir_lowering=False, debug=False,
                   num_devices=N_CORES)

    xT = nc.declare_dram_parameter("xT", [IN_F, N], f16, isOutput=False)
    Wd = nc.declare_dram_parameter("W", [IN_F, OUT_F], f16, isOutput=False)
    Wa = nc.declare_dram_parameter("Wa", [IN_F, 2 * H], f16, isOutput=False)
    onum = nc.declare_dram_parameter("onum", [H * 33, N], f32, isOutput=True)

    with tile.TileContext(nc) as tc:
      with (
        tc.tile_pool(name="const", bufs=1) as cpool,
        tc.tile_pool(name="atile", bufs=8) as apool,
        tc.tile_pool(name="otile", bufs=2) as opool,
      ):
        # ---- load inputs over the three DMA-capable queues ----
        xT_sb = cpool.tile([IN_F, N], f16, tag="xT")
        Wa_sb = cpool.tile([IN_F, 2 * H], f16, tag="Wa")
        W_sb = cpool.tile([IN_F, OUT_F], f16, tag="W")
        nc.sync.dma_start(out=Wa_sb, in_=Wa[:])
        nc.sync.dma_start(out=xT_sb[:, 0:342], in_=xT[:, 0:342])
        nc.scalar.dma_start(out=xT_sb[:, 342:684], in_=xT[:, 342:684])
        nc.gpsimd.dma_start(out=xT_sb[:, 684:N], in_=xT[:, 684:N])
        nc.gpsimd.dma_start(out=W_sb, in_=Wd[:])

        # prime the ACT exp table before it's on the critical path
        warm = cpool.tile([1, 8], f32, tag="warm")
        nc.scalar.activation(out=warm, in_=Wa_sb[0:1, 0:8], func=AF.Exp)

        # ---- t columns: tc_ps[j, 4*jt + h] = t_h[128*jt + j] ----
        with tc.tile_pool(name="ps_tc", bufs=1, space="PSUM") as pstc:
            tc_ps = pstc.tile([128, H * NT], f32, tag="tc")
            for jt in range(NT):
                nc.tensor.matmul(tc_ps[:, H * jt:H * (jt + 1)],
                                 xT_sb[:, 128 * jt:128 * (jt + 1)],
                                 Wa_sb[:, H:2 * H], start=True, stop=True)
            etc = cpool.tile([128, H * NT], f32, tag="etc")
            nc.scalar.activation(out=etc, in_=tc_ps, func=AF.Exp)
            etc02 = cpool.tile([128, H * NT], f32, tag="etc02")
            nc.scalar.activation(out=etc02, in_=tc_ps, func=AF.Exp, scale=NEG)

        # ---- es08_b[h][p, i] = e^{-0.8 s_h[i]}: rank-1-weight PE broadcast
        # matmul (every weight column = W a_src[h], stride-0 AP) + ACT exp.
        # bufs=1 serializes heads through one 2-bank slot; later heads are
        # emitted interleaved behind the main loop's matmuls.
        pssb = tc.tile_pool(name="ps_sb", bufs=1, space="PSUM")
        sbpool = pssb.__enter__()
        es08_b = {}

        def emit_sbcast(h):
            sb_ps = sbpool.tile([128, N], f32, tag="sb")
            wcol = Wa_sb[:, h:h + 1].to_broadcast([IN_F, 128])
            for c in range(2):
                nc.tensor.matmul(sb_ps[:, 512 * c:512 * (c + 1)], wcol,
                                 xT_sb[:, 512 * c:512 * (c + 1)],
                                 start=True, stop=True)
            eb = cpool.tile([128, N], bf16, tag=f"es08b{h}")
            nc.scalar.activation(out=eb, in_=sb_ps, func=AF.Exp, scale=-0.8)
            es08_b[h] = eb

        emit_sbcast(0)

        # ---- weight tiles: wt[:, 132jt+33h : +32] = h_node, col 32 = 1s ----
        wt_all = cpool.tile([128, NT * 33 * H], bf16, tag="wt")
        wt_v = wt_all[:].rearrange("p (jt h c) -> p jt h c", h=H, c=33)
        nc.vector.memset(wt_v[:, :, :, 32:33], 1.0)
        with tc.tile_pool(name="ps_hn", bufs=1, space="PSUM") as pshn:
            hn_ps = pshn.tile([128, N], f32, tag="hn")
            for jt in range(NT):
                nc.tensor.matmul(hn_ps[:, 128 * jt:128 * (jt + 1)],
                                 xT_sb[:, 128 * jt:128 * (jt + 1)], W_sb,
                                 start=True, stop=True)
                nc.vector.tensor_copy(
                    out=wt_v[:, jt, :, 0:32],
                    in_=hn_ps[:, 128 * jt:128 * (jt + 1)].rearrange(
                        "p (h c) -> p h c", c=32))
        wts = [wt_all[:, 132 * jt:132 * (jt + 1)] for jt in range(NT)]

        emit_sbcast(1)

        # numerator + Z rows out; the idle ACT engine does the PSUM read
        def emit_out(h, oh):
            ocp = opool.tile([33, N], f32, tag="ocp")
            nc.scalar.copy(out=ocp, in_=oh)
            q = nc.sync if h % 2 == 0 else nc.gpsimd
            q.dma_start(out=onum[33 * h:33 * (h + 1), :], in_=ocp)

        with tc.tile_pool(name="ps_main", bufs=3, space="PSUM") as psmain:
            ohs = [None] * H
            for h in range(H):
                oh = psmain.tile([33, N], f32, tag="oh")
                ohs[h] = oh
                for jt in range(NT):
                    idx = H * jt + h
                    a_t = apool.tile([128, N], bf16, tag="at")
                    nc.vector.tensor_scalar(
                        out=a_t, in0=es08_b[h],
                        scalar1=etc02[:, idx:idx + 1],
                        scalar2=etc[:, idx:idx + 1],
                        op0=ALU.mult, op1=ALU.max)
                    for c in range(2):
                        nc.tensor.matmul(
                            oh[:, 512 * c:512 * (c + 1)],
                            wts[jt][:, 33 * h:33 * (h + 1)],
                            a_t[:, 512 * c:512 * (c + 1)],
                            start=(jt == 0), stop=(jt == NT - 1))
                if h + 2 < H:
                    emit_sbcast(h + 2)
                if h >= 1:
                    emit_out(h - 1, ohs[h - 1])
            emit_out(H - 1, ohs[H - 1])
        pssb.__exit__(None, None, None)

    nc.compile()
    return nc


def _get_nc():
    if "nc" not in _CACHE:
        _CACHE["nc"] = _build_nc()
    return _CACHE["nc"]


def _prep_in_maps(x, W, a_src, a_dst):
    x = np.asarray(x, dtype=np.float32)
    W = np.asarray(W, dtype=np.float32)
    a_src = np.asarray(a_src, dtype=np.float32)
    a_dst = np.asarray(a_dst, dtype=np.float32)

    a_ext = np.zeros((OUT_F, 2 * H), np.float32)
    for h in range(H):
        a_ext[h * HD:(h + 1) * HD, h] = a_src[h]
        a_ext[h * HD:(h + 1) * HD, H + h] = a_dst[h]
    Wa = (W @ a_ext).astype(np.float16)
    W16 = W.astype(np.float16)

    return [
        {"xT": np.ascontiguousarray(x[c].T.astype(np.float16)),
         "W": W16, "Wa": Wa}
        for c in range(N_CORES)
    ]


def kernel(x, W, a_src, a_dst):
    from concourse.bass_utils import run_bass_kernel_spmd

    nc = _get_nc()
    in_maps = _prep_in_maps(x, W, a_src, a_dst)
    res = run_bass_kernel_spmd(nc, in_maps, core_ids=list(range(N_CORES)))
    out = np.empty((N_CORES, N, OUT_F), np.float32)
    for c in range(N_CORES):
        o = res.results[c]["onum"].reshape(H, 33, N)
        out[c] = (o[:, 0:HD, :] / o[:, HD:HD + 1, :]).transpose(2, 0, 1) \
            .reshape(N, OUT_F)
    return np.ascontiguousarray(out)
